# revision 6
# baseline (speedup 1.0000x reference)
"""Trainium2 Bass kernel for nn_MixtureCDFFlow: invert the per-channel
mixture-of-Gaussians CDF via dyadic bisection + clamped Newton, then
-log(pdf) for the log-det. Batch-sharded over 8 NeuronCores.

Hardcoded problem: B=128, S=2048, D=16, K=8 (fp32).

Per core (32768 tokens), state s = a*x + b lives on partitions (k,d)=128,
tokens along the free dim, processed in two half-passes of 16384 tokens:
  bisection:  F-z = W1^T erf(s) - z  (fp32 matmul + exact hi/lo z split via
              fp32r matmuls), step s +- ac_i from sign(F-z) (ties step up,
              matching the reference).
  newton:     dv = (F-z) * recip_fast(-pdf/a - eps), clamped per iteration
              to a dyadic cap schedule summing to the final bracket
              half-width, so saturated-CDF runaways (z>=0.99999988, forced
              via z:=2 on host) land exactly on the bracket top = ub0 like
              the reference's bisection does.
  outputs:    x = (s - b)/a, nld = -Ln(pdf); Ln(0) = -inf reproduces the
              reference's inf on runaway elements.
"""
import sys

import numpy as np

sys.path.insert(0, "/opt/trn_rl_repo")

import concourse.tile as tile  # noqa: E402
from concourse import bacc, mybir  # noqa: E402
from concourse.bass_utils import run_bass_kernel_spmd  # noqa: E402
from concourse.masks import make_identity  # noqa: E402

f32 = np.float32
AF = mybir.ActivationFunctionType
ALU = mybir.AluOpType
DT = mybir.dt

B, S, D, K = 128, 2048, 16, 8
NCORES = 8
BSH = B // NCORES
NTOK = BSH * S               # 32768 tokens/core
NH = NTOK // 2               # half-pass tokens
TC = 2048                    # chunk free size
NCHUNK = NH // TC
NQ = TC // 512
AH = NH // 128               # tokens per partition in L/OF layout (=128)

N_BIS = 14
N_NEWT = 4
CAP_FRAC = (0.5, 0.25, 0.125, 0.125)
EPS_PDF = -1e-9
RUN_THRESH = f32(1.0 - 2.0 * 2.0 ** -24)  # 0.99999988

_SQRT2 = f32(np.sqrt(2.0))
_MAGIC = f32(12582912.0)      # 1.5 * 2^23: RNE round-to-int trick

# SC scalar-const columns
C_S0C = 0
C_FCC = 1
C_AC = 2
C_NAC2 = C_AC + N_BIS
C_CAP = C_NAC2 + N_BIS
C_NCAP = C_CAP + N_NEWT
C_BOUT = C_NCAP + N_NEWT
C_INVA = C_BOUT + 1
NCOL = C_INVA + 1

# CONSTF packed fp32 columns: W1 | V2 | B0(rows<16) | V3 | SC
F_W1, F_V2, F_B0, F_V3, F_SC = 0, 128, 256, 384, 400
CF_COLS = 400 + NCOL
# CONSTR packed fp32r columns: Z2(rows<32) | EPS(row 0) | ONES(row 0)
R_Z2, R_EPS, R_ONES = 0, 128, 256
CR_COLS = 256 + TC


def _erf64(v):
    try:
        from scipy.special import erf
        return erf(v.astype(np.float64))
    except Exception:
        import math
        return np.vectorize(math.erf)(v.astype(np.float64))


def _prep(logits, mu, logstd):
    l = np.asarray(logits, f32)
    e = np.exp((l - l.max()).astype(f32)).astype(f32)
    w = (e / e.sum().astype(f32)).astype(f32)
    scale = np.exp(np.asarray(logstd, f32)).astype(f32)
    istd = (f32(1.0) / scale).astype(f32)
    mu = np.asarray(mu, f32)
    a = (istd / _SQRT2).astype(f32)
    b = (-mu * a).astype(f32)
    maxscales = scale.sum(0, dtype=f32)
    lb0 = (mu - f32(10.0) * maxscales).min(0).astype(f32)
    ub0 = (mu + f32(10.0) * maxscales).max(0).astype(f32)
    C = ((lb0 + ub0) * f32(0.5)).astype(f32)
    W = (ub0 - lb0).astype(f32)

    W1 = np.zeros((128, 128), f32)
    V2 = np.zeros((128, 128), f32)
    V3 = np.zeros((128, D), f32)
    for k in range(K):
        rs = slice(k * D, (k + 1) * D)
        V3[rs, :] = np.diag(w[k] * istd[k] / f32(2.0 * np.sqrt(2.0)))
        for kp in range(K):
            cs = slice(kp * D, (kp + 1) * D)
            W1[rs, cs] = np.diag((w[k] * f32(0.5)) * np.ones(D, f32))
            V2[rs, cs] = np.diag(-(w[k] * f32(0.5)) * istd[k] / istd[kp])
    Z2 = np.zeros((128, 128), f32)
    for hl in range(2):
        for kp in range(K):
            Z2[hl * D:(hl + 1) * D, kp * D:(kp + 1) * D] = -np.eye(D, dtype=f32)

    ac = [(a * (W * f32(2.0 ** (-(i + 2)))).astype(f32)).astype(f32).reshape(128)
          for i in range(N_BIS)]
    acl = ac[N_BIS - 1]
    s0 = (a * C + b).astype(f32).reshape(128)

    Fc = np.zeros(D, f32)
    for k in range(K):
        Fc += (w[k] * f32(0.5) * _erf64((a[k] * C + b[k])).astype(f32)).astype(f32)

    B0 = np.zeros((16, 128), f32)
    for kp in range(K):
        B0[:, kp * D:(kp + 1) * D] = np.diag(
            (-f32(2.0)) * ac[0][kp * D:(kp + 1) * D])

    SC = np.zeros((128, NCOL), f32)
    SC[:, C_S0C] = s0 + ac[0]
    SC[:, C_FCC] = np.tile(Fc, 8)
    for i in range(N_BIS):
        SC[:, C_AC + i] = ac[i]
        SC[:, C_NAC2 + i] = f32(-2.0) * ac[i]
    for j in range(N_NEWT):
        SC[:, C_CAP + j] = acl * f32(CAP_FRAC[j])
        SC[:, C_NCAP + j] = -(acl * f32(CAP_FRAC[j]))
    SC[0:D, C_BOUT] = b[0]
    SC[0:D, C_INVA] = (f32(1.0) / a[0]).astype(f32)

    CONSTF = np.zeros((128, CF_COLS), f32)
    CONSTF[:, F_W1:F_W1 + 128] = W1
    CONSTF[:, F_V2:F_V2 + 128] = V2
    CONSTF[0:16, F_B0:F_B0 + 128] = B0
    CONSTF[:, F_V3:F_V3 + D] = V3
    CONSTF[:, F_SC:F_SC + NCOL] = SC

    CONSTR = np.zeros((128, CR_COLS), f32)
    for rep in range(4):
        CONSTR[rep * 32:(rep + 1) * 32, R_Z2:R_Z2 + 128] = Z2[0:32]
    CONSTR[0, R_EPS:R_EPS + 128] = EPS_PDF
    CONSTR[0, R_ONES:R_ONES + TC] = 1.0
    return dict(CONSTF=CONSTF, CONSTR=CONSTR)


def _build_nc():
    nc = bacc.Bacc()
    z_in = nc.dram_tensor("z_in", [NTOK, D], DT.float32, kind="ExternalInput")
    cf_d = nc.dram_tensor("CONSTF", [128, CF_COLS], DT.float32, kind="ExternalInput")
    cr_d = nc.dram_tensor("CONSTR", [128, CR_COLS], DT.float32, kind="ExternalInput")
    x_out = nc.dram_tensor("x_out", [NTOK, D], DT.float32, kind="ExternalOutput")
    nld_out = nc.dram_tensor("nld_out", [NTOK, D], DT.float32, kind="ExternalOutput")

    with tile.TileContext(nc) as tc:
        with (
            tc.tile_pool(name="const", bufs=1) as cp,
            tc.tile_pool(name="state", bufs=1) as st,
            tc.tile_pool(name="work", bufs=2) as wk,
            tc.tile_pool(name="once", bufs=1) as w1p,
            tc.tile_pool(name="ps1", bufs=2, space="PSUM") as ps1,
            tc.tile_pool(name="ps2", bufs=2, space="PSUM") as ps2,
            tc.tile_pool(name="pst", bufs=2, space="PSUM") as pst,
        ):
            CF = cp.tile([128, CF_COLS], DT.float32)
            CR = cp.tile([128, CR_COLS], DT.float32r)
            nc.gpsimd.dma_start(CF, cf_d[:, :])
            nc.gpsimd.dma_start(CR, cr_d[:, :])   # cast fp32 -> fp32r
            ident = cp.tile([128, 128], DT.float32)
            make_identity(nc, ident)

            W1 = CF[:, F_W1:F_W1 + 128]
            V2 = CF[:, F_V2:F_V2 + 128]
            B0 = CF[0:16, F_B0:F_B0 + 128]
            V3 = CF[:, F_V3:F_V3 + D]
            SCc = lambda col: CF[:, F_SC + col:F_SC + col + 1]
            SCc16 = lambda col: CF[0:D, F_SC + col:F_SC + col + 1]
            Z2rep = lambda c: CR[(c % 3) * 32:(c % 3) * 32 + 32, R_Z2:R_Z2 + 128]
            EPS = CR[0:1, R_EPS:R_EPS + 128]
            ONES = CR[0:1, R_ONES:R_ONES + TC]

            for h in range(2):
                zview = z_in[h * NH:(h + 1) * NH, :].rearrange(
                    "(p a) d -> p (a d)", p=128)
                L = w1p.tile([128, AH * D], DT.float32, tag="L")
                nc.sync.dma_start(L, zview)
                Lv = L[:, :].rearrange("p (a d) -> p a d", d=D)

                # transpose in (tau = a*128 + p), z-0.5, fold bisection
                # step 0, and build the fp32r hi/lo z split -- all in
                # base-partition-0 staging, placed into zs by cast-DMA.
                s = st.tile([128, NH], DT.float32, tag="s")
                zs = [st.tile([96, TC], DT.float32r, tag=f"zs{t}",
                              name=f"zs{t}_{h}") for t in range(3)]
                zhc = w1p.tile([D, TC], DT.float32, tag="OFX")
                zlc = w1p.tile([D, TC], DT.float32, tag="OFN")
                for blk in range(NH // 512):
                    c, qq = blk // 4, blk % 4
                    ptz = pst.tile([16, 512], DT.float32, tag="ptz")
                    for q in range(4):
                        nc.tensor.transpose(
                            ptz[:, q * 128:(q + 1) * 128],
                            Lv[:, blk * 4 + q, :], ident)
                    zc0 = wk.tile([D, 512], DT.float32, tag="TR")
                    nc.vector.tensor_scalar(zc0, ptz, 0.5, None, ALU.subtract)
                    m0 = wk.tile([D, 512], DT.float32, tag="TA")
                    nc.vector.tensor_scalar(
                        m0, zc0, CF[0:D, F_SC + C_FCC:F_SC + C_FCC + 1],
                        None, ALU.is_lt)
                    p0 = ps1.tile([128, 512], DT.float32, tag="p1")
                    nc.tensor.matmul(p0, B0, m0, start=True, stop=True)
                    nc.vector.tensor_scalar(
                        s[:, blk * 512:(blk + 1) * 512], p0,
                        SCc(C_S0C), None, ALU.add)
                    lsl = zlc[:, qq * 512:(qq + 1) * 512]
                    hsl = zhc[:, qq * 512:(qq + 1) * 512]
                    nc.vector.tensor_scalar(
                        lsl, zc0, 4096.0, float(_MAGIC), ALU.mult, ALU.add)
                    nc.vector.tensor_scalar(
                        hsl, lsl, float(_MAGIC), 2.0 ** -12,
                        ALU.subtract, ALU.mult)
                    nc.vector.tensor_tensor(lsl, zc0, hsl, ALU.subtract)
                    if qq == 3:
                        r0 = (c % 3) * 32
                        nc.gpsimd.dma_start(zs[c // 3][r0:r0 + D, :], zhc)
                        nc.gpsimd.dma_start(zs[c // 3][r0 + D:r0 + 32, :], zlc)

                # dyadic bisection
                for i in range(1, N_BIS):
                    for c in range(NCHUNK):
                        scs = s[:, c * TC:(c + 1) * TC]
                        zmov = zs[c // 3][(c % 3) * 32:(c % 3) * 32 + 32, :]
                        E = wk.tile([128, TC], DT.float32, tag="TA")
                        nc.scalar.activation(E, scs, AF.Erf)
                        sgn = wk.tile([128, TC], DT.float32, tag="TB")
                        for q in range(NQ):
                            sl = slice(q * 512, (q + 1) * 512)
                            p1 = ps1.tile([128, 512], DT.float32, tag="p1")
                            nc.tensor.matmul(p1, W1, E[:, sl],
                                             start=True, stop=False)
                            nc.tensor.matmul(p1, Z2rep(c), zmov[:, sl],
                                             start=False, stop=True)
                            nc.scalar.activation(sgn[:, sl], p1, AF.Sign)
                        nc.vector.tensor_scalar(
                            sgn, sgn, 0.0, SCc(C_NAC2 + i), ALU.max, ALU.mult)
                        nc.vector.affine_then_add(
                            scs, sgn, scs, 1.0, SCc(C_AC + i))

                # Newton with dyadic caps
                for j in range(N_NEWT):
                    for c in range(NCHUNK):
                        scs = s[:, c * TC:(c + 1) * TC]
                        zmov = zs[c // 3][(c % 3) * 32:(c % 3) * 32 + 32, :]
                        E = wk.tile([128, TC], DT.float32, tag="TA")
                        nc.scalar.activation(E, scs, AF.Erf)
                        G = wk.tile([128, TC], DT.float32, tag="TB")
                        nc.scalar.activation(G, scs, AF.Derivative_Erf)
                        R = wk.tile([128, TC], DT.float32, tag="TR")
                        for q in range(NQ):
                            sl = slice(q * 512, (q + 1) * 512)
                            p1 = ps1.tile([128, 512], DT.float32, tag="p1")
                            nc.tensor.matmul(p1, W1, E[:, sl],
                                             start=True, stop=False)
                            nc.tensor.matmul(p1, Z2rep(c), zmov[:, sl],
                                             start=False, stop=True)
                            p2 = ps2.tile([128, 512], DT.float32, tag="p2")
                            nc.tensor.matmul(p2, V2, G[:, sl],
                                             start=True, stop=False)
                            nc.tensor.matmul(p2, EPS, ONES[:, sl],
                                             start=False, stop=True)
                            nc.vector.reciprocal_approx_fast(R[:, sl], p2)
                            nc.vector.tensor_tensor(R[:, sl], p1, R[:, sl],
                                                    ALU.mult)
                        nc.vector.tensor_scalar(
                            R, R, SCc(C_CAP + j), SCc(C_NCAP + j),
                            ALU.min, ALU.max)
                        nc.vector.tensor_tensor(scs, scs, R, ALU.add)

                # logdet + outputs (transpose back to token-major)
                OFX = w1p.tile([128, AH * D], DT.float32, tag="OFX")
                OFN = w1p.tile([128, AH * D], DT.float32, tag="OFN")
                for c in range(NCHUNK):
                    scs = s[:, c * TC:(c + 1) * TC]
                    G = wk.tile([128, TC], DT.float32, tag="TA")
                    nc.scalar.activation(G, scs, AF.Derivative_Erf)
                    nld = wk.tile([D, TC], DT.float32, tag="TB")
                    for q in range(NQ):
                        sl = slice(q * 512, (q + 1) * 512)
                        p3 = ps1.tile([16, 512], DT.float32, tag="p1")
                        nc.tensor.matmul(p3, V3, G[:, sl], start=True, stop=True)
                        nc.scalar.activation(nld[:, sl], p3, AF.Ln)
                    xo = wk.tile([D, TC], DT.float32, tag="TR")
                    nc.vector.tensor_scalar(
                        xo, scs[0:D, :], SCc16(C_BOUT), SCc16(C_INVA),
                        ALU.subtract, ALU.mult)
                    for hf in range(2):
                        pox = pst.tile([128, 128], DT.float32, tag="pot")
                        for lt in range(8):
                            tt = hf * 8 + lt
                            nc.tensor.transpose(
                                pox[:, lt * D:(lt + 1) * D],
                                xo[:, tt * 128:(tt + 1) * 128],
                                ident[0:D, 0:D])
                        o0 = c * 256 + hf * 128
                        nc.vector.tensor_copy(OFX[:, o0:o0 + 128], pox)
                        pon = pst.tile([128, 128], DT.float32, tag="pot")
                        for lt in range(8):
                            tt = hf * 8 + lt
                            nc.tensor.transpose(
                                pon[:, lt * D:(lt + 1) * D],
                                nld[:, tt * 128:(tt + 1) * 128],
                                ident[0:D, 0:D])
                        nc.vector.tensor_scalar(
                            OFN[:, o0:o0 + 128], pon, -1.0, None, ALU.mult)
                for od, OF in ((x_out, OFX), (nld_out, OFN)):
                    oview = od[h * NH:(h + 1) * NH, :].rearrange(
                        "(p a) d -> p (a d)", p=128)
                    nc.sync.dma_start(oview, OF)
    nc.finalize()
    return nc


_NC_CACHE = {}


def _get_nc():
    if "nc" not in _NC_CACHE:
        _NC_CACHE["nc"] = _build_nc()
    return _NC_CACHE["nc"]


def kernel(z, logits, mu, logstd):
    z = np.asarray(z, f32)
    consts = _prep(logits, mu, logstd)
    zp = np.where(z >= RUN_THRESH, f32(2.0), z).astype(f32)

    in_maps = []
    for core in range(NCORES):
        zi = np.ascontiguousarray(zp[core * BSH:(core + 1) * BSH].reshape(NTOK, D))
        in_maps.append(dict(z_in=zi, **consts))

    res = run_bass_kernel_spmd(_get_nc(), in_maps, core_ids=list(range(NCORES)))
    x = np.empty((B, S, D), f32)
    nld = np.empty((B, S, D), f32)
    for core in range(NCORES):
        r = res.results[core]
        x[core * BSH:(core + 1) * BSH] = r["x_out"].reshape(BSH, S, D)
        nld[core * BSH:(core + 1) * BSH] = r["nld_out"].reshape(BSH, S, D)
    nld = np.where(z >= RUN_THRESH, np.float32(np.inf), nld).astype(f32)
    return x, nld


# revision 8
# speedup vs baseline: 33.5389x; 33.5389x over previous
"""Trainium2 Bass kernel for nn_MixtureCDFFlow: invert the per-channel
mixture-of-Gaussians CDF via dyadic bisection + clamped Newton, then
-log(pdf) for the log-det. Batch-sharded over 8 NeuronCores.

Hardcoded problem: B=128, S=2048, D=16, K=8 (fp32).

Per core (32768 tokens), state s = a*x + b lives on partitions (k,d)=128,
tokens along the free dim, processed in two half-passes of 16384 tokens:
  bisection:  F-z = W1^T erf(s) - z  (fp32 matmul + exact hi/lo z split via
              fp32r matmuls), step s +- ac_i from sign(F-z) (ties step up,
              matching the reference).
  newton:     dv = (F-z) * recip_fast(-pdf/a - eps), clamped per iteration
              to a dyadic cap schedule summing to the final bracket
              half-width, so saturated-CDF runaways (z>=0.99999988, forced
              via z:=2 on host) land exactly on the bracket top = ub0 like
              the reference's bisection does.
  outputs:    x = (s - b)/a, nld = -Ln(pdf); Ln(0) = -inf reproduces the
              reference's inf on runaway elements.
"""
import sys

import numpy as np

sys.path.insert(0, "/opt/trn_rl_repo")

import concourse.tile as tile  # noqa: E402
from concourse import bacc, mybir  # noqa: E402
from concourse.bass_utils import run_bass_kernel_spmd  # noqa: E402
from concourse.masks import make_identity  # noqa: E402

f32 = np.float32
AF = mybir.ActivationFunctionType
ALU = mybir.AluOpType
DT = mybir.dt

B, S, D, K = 128, 2048, 16, 8
NCORES = 8
BSH = B // NCORES
NTOK = BSH * S               # 32768 tokens/core
NH = NTOK // 2               # half-pass tokens
TC = 2048                    # chunk free size
NCHUNK = NH // TC
NQ = TC // 512
AH = NH // 128               # tokens per partition in L/OF layout (=128)

N_BIS = 14
N_NEWT = 4
CAP_FRAC = (0.5, 0.25, 0.125, 0.125)
EPS_PDF = -1e-9
RUN_THRESH = f32(1.0 - 2.0 * 2.0 ** -24)  # 0.99999988

_SQRT2 = f32(np.sqrt(2.0))
_MAGIC = f32(12582912.0)      # 1.5 * 2^23: RNE round-to-int trick

# SC scalar-const columns
C_S0C = 0
C_FCC = 1
C_AC = 2
C_NAC2 = C_AC + N_BIS
C_CAP = C_NAC2 + N_BIS
C_NCAP = C_CAP + N_NEWT
C_BOUT = C_NCAP + N_NEWT
C_INVA = C_BOUT + 1
NCOL = C_INVA + 1

# CONSTF packed fp32 columns: W1 | V2 | B0(rows<16) | V3 | SC
F_W1, F_V2, F_B0, F_V3, F_SC = 0, 128, 256, 384, 400
CF_COLS = 400 + NCOL
# CONSTR packed fp32r columns: Z2(rows<32) | EPS(row 0) | ONES(row 0) | B0
R_Z2, R_EPS, R_ONES = 0, 128, 256
R_B0 = 256 + TC
CR_COLS = R_B0 + 128


def _erf64(v):
    try:
        from scipy.special import erf
        return erf(v.astype(np.float64))
    except Exception:
        import math
        return np.vectorize(math.erf)(v.astype(np.float64))


def _prep(logits, mu, logstd):
    l = np.asarray(logits, f32)
    e = np.exp((l - l.max()).astype(f32)).astype(f32)
    w = (e / e.sum().astype(f32)).astype(f32)
    scale = np.exp(np.asarray(logstd, f32)).astype(f32)
    istd = (f32(1.0) / scale).astype(f32)
    mu = np.asarray(mu, f32)
    a = (istd / _SQRT2).astype(f32)
    b = (-mu * a).astype(f32)
    maxscales = scale.sum(0, dtype=f32)
    lb0 = (mu - f32(10.0) * maxscales).min(0).astype(f32)
    ub0 = (mu + f32(10.0) * maxscales).max(0).astype(f32)
    C = ((lb0 + ub0) * f32(0.5)).astype(f32)
    W = (ub0 - lb0).astype(f32)

    W1 = np.zeros((128, 128), f32)
    V2 = np.zeros((128, 128), f32)
    V3 = np.zeros((128, D), f32)
    for k in range(K):
        rs = slice(k * D, (k + 1) * D)
        V3[rs, :] = np.diag(w[k] * istd[k] / f32(2.0 * np.sqrt(2.0)))
        for kp in range(K):
            cs = slice(kp * D, (kp + 1) * D)
            W1[rs, cs] = np.diag((w[k] * f32(0.5)) * np.ones(D, f32))
            V2[rs, cs] = np.diag(-(w[k] * f32(0.5)) * istd[k] / istd[kp])
    Z2 = np.zeros((128, 128), f32)
    for hl in range(2):
        for kp in range(K):
            Z2[hl * D:(hl + 1) * D, kp * D:(kp + 1) * D] = -np.eye(D, dtype=f32)

    ac = [(a * (W * f32(2.0 ** (-(i + 2)))).astype(f32)).astype(f32).reshape(128)
          for i in range(N_BIS)]
    acl = ac[N_BIS - 1]
    s0 = (a * C + b).astype(f32).reshape(128)

    Fc = np.zeros(D, f32)
    for k in range(K):
        Fc += (w[k] * f32(0.5) * _erf64((a[k] * C + b[k])).astype(f32)).astype(f32)

    B0 = np.zeros((16, 128), f32)
    for kp in range(K):
        B0[:, kp * D:(kp + 1) * D] = np.diag(
            (-f32(2.0)) * ac[0][kp * D:(kp + 1) * D])

    SC = np.zeros((128, NCOL), f32)
    SC[:, C_S0C] = s0 + ac[0]
    SC[:, C_FCC] = np.tile(Fc, 8)
    for i in range(N_BIS):
        SC[:, C_AC + i] = ac[i]
        SC[:, C_NAC2 + i] = f32(-2.0) * ac[i]
    for j in range(N_NEWT):
        SC[:, C_CAP + j] = acl * f32(CAP_FRAC[j])
        SC[:, C_NCAP + j] = -(acl * f32(CAP_FRAC[j]))
    SC[0:D, C_BOUT] = b[0]
    SC[0:D, C_INVA] = (f32(1.0) / a[0]).astype(f32)

    CONSTF = np.zeros((128, CF_COLS), f32)
    CONSTF[:, F_W1:F_W1 + 128] = W1
    CONSTF[:, F_V2:F_V2 + 128] = V2
    CONSTF[0:16, F_B0:F_B0 + 128] = B0
    CONSTF[:, F_V3:F_V3 + D] = V3
    CONSTF[:, F_SC:F_SC + NCOL] = SC

    CONSTR = np.zeros((128, CR_COLS), f32)
    for rep in range(4):
        CONSTR[rep * 32:(rep + 1) * 32, R_Z2:R_Z2 + 128] = Z2[0:32]
    CONSTR[0, R_EPS:R_EPS + 128] = EPS_PDF
    CONSTR[0, R_ONES:R_ONES + TC] = 1.0
    CONSTR[0:16, R_B0:R_B0 + 128] = B0
    return dict(CONSTF=CONSTF, CONSTR=CONSTR)


def _build_nc(nrep=1):
    nc = bacc.Bacc()
    z_in = nc.dram_tensor("z_in", [NTOK, D], DT.float32, kind="ExternalInput")
    cf_d = nc.dram_tensor("CONSTF", [128, CF_COLS], DT.float32, kind="ExternalInput")
    cr_d = nc.dram_tensor("CONSTR", [128, CR_COLS], DT.float32, kind="ExternalInput")
    x_out = nc.dram_tensor("x_out", [NTOK, D], DT.float32, kind="ExternalOutput")
    nld_out = nc.dram_tensor("nld_out", [NTOK, D], DT.float32, kind="ExternalOutput")

    with tile.TileContext(nc) as tc:
        with (
            tc.tile_pool(name="const", bufs=1) as cp,
            tc.tile_pool(name="state", bufs=1) as st,
            tc.tile_pool(name="work", bufs=2) as wk,
            tc.tile_pool(name="once", bufs=1) as w1p,
            tc.tile_pool(name="ps1", bufs=2, space="PSUM") as ps1,
            tc.tile_pool(name="ps2", bufs=2, space="PSUM") as ps2,
            tc.tile_pool(name="pst", bufs=2, space="PSUM") as pst,
        ):
            CF = cp.tile([128, CF_COLS], DT.float32)
            CR = cp.tile([128, CR_COLS], DT.float32r)
            nc.gpsimd.dma_start(CF, cf_d[:, :])
            nc.gpsimd.dma_start(CR, cr_d[:, :])   # cast fp32 -> fp32r
            ident = cp.tile([128, 128], DT.float32)
            make_identity(nc, ident)

            W1 = CF[:, F_W1:F_W1 + 128]
            V2 = CF[:, F_V2:F_V2 + 128]
            B0 = CF[0:16, F_B0:F_B0 + 128]
            V3 = CF[:, F_V3:F_V3 + D]
            SCc = lambda col: CF[:, F_SC + col:F_SC + col + 1]
            SCc16 = lambda col: CF[0:D, F_SC + col:F_SC + col + 1]
            Z2rep = lambda c: CR[(c % 3) * 32:(c % 3) * 32 + 32, R_Z2:R_Z2 + 128]
            EPS = CR[0:1, R_EPS:R_EPS + 128]
            ONES = CR[0:1, R_ONES:R_ONES + TC]

            for h in [hh for _ in range(nrep) for hh in range(2)]:
                zview = z_in[h * NH:(h + 1) * NH, :].rearrange(
                    "(p a) d -> p (a d)", p=128)
                L = w1p.tile([128, AH * D], DT.float32, tag="L")
                nc.sync.dma_start(L, zview)
                Lv = L[:, :].rearrange("p (a d) -> p a d", d=D)

                # transpose in (tau = a*128 + p), z-0.5, fold bisection
                # step 0, and build the fp32r hi/lo z split -- all in
                # base-partition-0 staging, placed into zs by cast-DMA.
                s = st.tile([128, NH], DT.float32, tag="s")
                zs = [st.tile([96, TC], DT.float32r, tag=f"zs{t}",
                              name=f"zs{t}_{h}") for t in range(3)]
                zhc = w1p.tile([D, TC], DT.float32, tag="OFX")
                zlc = w1p.tile([D, TC], DT.float32, tag="OFN")
                zcc = w1p.tile([D, TC], DT.float32, tag="zcc")
                for blk in range(NH // 512):
                    c, qq = blk // 4, blk % 4
                    ptz = pst.tile([16, 512], DT.float32, tag="ptz")
                    for q in range(4):
                        nc.tensor.transpose(
                            ptz[:, q * 128:(q + 1) * 128],
                            Lv[:, blk * 4 + q, :], ident)
                    zc0 = wk.tile([D, 512], DT.float32, tag="TR")
                    nc.vector.tensor_scalar(zc0, ptz, 0.5, None, ALU.subtract)
                    m0 = wk.tile([D, 512], DT.float32, tag="TA")
                    nc.vector.tensor_scalar(
                        m0, zc0, CF[0:D, F_SC + C_FCC:F_SC + C_FCC + 1],
                        None, ALU.is_lt)
                    p0 = ps1.tile([128, 512], DT.float32, tag="p1")
                    nc.tensor.matmul(p0, B0, m0, start=True, stop=True)
                    nc.vector.tensor_scalar(
                        s[:, blk * 512:(blk + 1) * 512], p0,
                        SCc(C_S0C), None, ALU.add)
                    nc.vector.tensor_copy(
                        zcc[:, qq * 512:(qq + 1) * 512], zc0)
                    if qq == 3:
                        nc.vector.tensor_scalar(
                            zlc, zcc, 4096.0, float(_MAGIC), ALU.mult, ALU.add)
                        nc.vector.tensor_scalar(
                            zhc, zlc, float(_MAGIC), 2.0 ** -12,
                            ALU.subtract, ALU.mult)
                        nc.vector.tensor_tensor(zlc, zcc, zhc, ALU.subtract)
                        r0 = (c % 3) * 32
                        nc.gpsimd.dma_start(zs[c // 3][r0:r0 + D, :], zhc)
                        nc.gpsimd.dma_start(zs[c // 3][r0 + D:r0 + 32, :], zlc)

                # dyadic bisection
                for i in range(1, N_BIS):
                    for c in range(NCHUNK):
                        scs = s[:, c * TC:(c + 1) * TC]
                        zmov = zs[c // 3][(c % 3) * 32:(c % 3) * 32 + 32, :]
                        E = wk.tile([128, TC], DT.float32, tag="TA")
                        nc.scalar.activation(E, scs, AF.Erf)
                        sgn = wk.tile([128, TC], DT.float32, tag="TB")
                        for q in range(NQ):
                            sl = slice(q * 512, (q + 1) * 512)
                            p1 = ps1.tile([128, 512], DT.float32, tag="p1")
                            nc.tensor.matmul(p1, W1, E[:, sl],
                                             start=True, stop=False)
                            nc.tensor.matmul(p1, Z2rep(c), zmov[:, sl],
                                             start=False, stop=True)
                            nc.scalar.activation(sgn[:, sl], p1, AF.Sign)
                        nc.vector.tensor_scalar(
                            sgn, sgn, 0.0, SCc(C_NAC2 + i), ALU.max, ALU.mult)
                        nc.vector.affine_then_add(
                            scs, sgn, scs, 1.0, SCc(C_AC + i))

                # Newton with dyadic caps
                for j in range(N_NEWT):
                    for c in range(NCHUNK):
                        scs = s[:, c * TC:(c + 1) * TC]
                        zmov = zs[c // 3][(c % 3) * 32:(c % 3) * 32 + 32, :]
                        E = wk.tile([128, TC], DT.float32, tag="TA")
                        nc.scalar.activation(E, scs, AF.Erf)
                        G = wk.tile([128, TC], DT.float32, tag="TB")
                        nc.scalar.activation(G, scs, AF.Derivative_Erf)
                        R = wk.tile([128, TC], DT.float32, tag="TR")
                        for q in range(NQ):
                            sl = slice(q * 512, (q + 1) * 512)
                            p1 = ps1.tile([128, 512], DT.float32, tag="p1")
                            nc.tensor.matmul(p1, W1, E[:, sl],
                                             start=True, stop=False)
                            nc.tensor.matmul(p1, Z2rep(c), zmov[:, sl],
                                             start=False, stop=True)
                            p2 = ps2.tile([128, 512], DT.float32, tag="p2")
                            nc.tensor.matmul(p2, V2, G[:, sl],
                                             start=True, stop=False)
                            nc.tensor.matmul(p2, EPS, ONES[:, sl],
                                             start=False, stop=True)
                            nc.vector.reciprocal_approx_fast(R[:, sl], p2)
                            nc.vector.tensor_tensor(R[:, sl], p1, R[:, sl],
                                                    ALU.mult)
                        nc.vector.tensor_scalar(
                            R, R, SCc(C_CAP + j), SCc(C_NCAP + j),
                            ALU.min, ALU.max)
                        nc.gpsimd.tensor_tensor(scs, scs, R, ALU.add)

                # logdet + outputs (transpose back to token-major)
                OFX = w1p.tile([128, AH * D], DT.float32, tag="OFX")
                OFN = w1p.tile([128, AH * D], DT.float32, tag="OFN")
                for c in range(NCHUNK):
                    scs = s[:, c * TC:(c + 1) * TC]
                    G = wk.tile([128, TC], DT.float32, tag="TA")
                    nc.scalar.activation(G, scs, AF.Derivative_Erf)
                    nld = wk.tile([D, TC], DT.float32, tag="TB")
                    for q in range(NQ):
                        sl = slice(q * 512, (q + 1) * 512)
                        p3 = ps1.tile([16, 512], DT.float32, tag="p1")
                        nc.tensor.matmul(p3, V3, G[:, sl], start=True, stop=True)
                        nc.scalar.activation(nld[:, sl], p3, AF.Ln)
                    xo = wk.tile([D, TC], DT.float32, tag="TR")
                    nc.vector.tensor_scalar(
                        xo, scs[0:D, :], SCc16(C_BOUT), SCc16(C_INVA),
                        ALU.subtract, ALU.mult)
                    for hf in range(2):
                        pox = pst.tile([128, 128], DT.float32, tag="pot")
                        for lt in range(8):
                            tt = hf * 8 + lt
                            nc.tensor.transpose(
                                pox[:, lt * D:(lt + 1) * D],
                                xo[:, tt * 128:(tt + 1) * 128],
                                ident[0:D, 0:D])
                        o0 = c * 256 + hf * 128
                        nc.vector.tensor_copy(OFX[:, o0:o0 + 128], pox)
                        pon = pst.tile([128, 128], DT.float32, tag="pot")
                        for lt in range(8):
                            tt = hf * 8 + lt
                            nc.tensor.transpose(
                                pon[:, lt * D:(lt + 1) * D],
                                nld[:, tt * 128:(tt + 1) * 128],
                                ident[0:D, 0:D])
                        nc.vector.tensor_scalar(
                            OFN[:, o0:o0 + 128], pon, -1.0, None, ALU.mult)
                for od, OF in ((x_out, OFX), (nld_out, OFN)):
                    oview = od[h * NH:(h + 1) * NH, :].rearrange(
                        "(p a) d -> p (a d)", p=128)
                    nc.sync.dma_start(oview, OF)
    nc.finalize()
    return nc


_NC_CACHE = {}


def _get_nc():
    if "nc" not in _NC_CACHE:
        _NC_CACHE["nc"] = _build_nc()
    return _NC_CACHE["nc"]


def kernel(z, logits, mu, logstd):
    z = np.asarray(z, f32)
    consts = _prep(logits, mu, logstd)
    zp = np.where(z >= RUN_THRESH, f32(2.0), z).astype(f32)

    in_maps = []
    for core in range(NCORES):
        zi = np.ascontiguousarray(zp[core * BSH:(core + 1) * BSH].reshape(NTOK, D))
        in_maps.append(dict(z_in=zi, **consts))

    res = run_bass_kernel_spmd(_get_nc(), in_maps, core_ids=list(range(NCORES)))
    x = np.empty((B, S, D), f32)
    nld = np.empty((B, S, D), f32)
    for core in range(NCORES):
        r = res.results[core]
        x[core * BSH:(core + 1) * BSH] = r["x_out"].reshape(BSH, S, D)
        nld[core * BSH:(core + 1) * BSH] = r["nld_out"].reshape(BSH, S, D)
    nld = np.where(z >= RUN_THRESH, np.float32(np.inf), nld).astype(f32)
    return x, nld
